# revision 2
# baseline (speedup 1.0000x reference)
"""Trainium2 Bass kernel for nn_CNNNet (TextCNN: embedding gather -> 18
conv filters (widths 2..19) -> max-over-time -> linear 18->2).

Strategy
--------
Data-parallel over batch: 256 rows -> 32 per NeuronCore x 8 cores.
Per core:
  * fp16 embedding table (padding row 32000 zeroed) stays in HBM; tokens are
    gathered with the SWDGE `dma_gather(transpose=True)` custom instruction,
    which lands each batch-row directly in [128 emb-dims, tokens] SBUF layout
    (the matmul rhs layout) at ~256B/token descriptors across 16 SDMA engines.
  * The 18 convolutions are evaluated as 25 accumulating TensorE matmuls per
    batch row using a time-folded weight packing: output column n covers the
    7 time steps t = 7n + r (r = 0..6); PSUM partition m = 7*j + r holds
    conv_j[7n + r].  For tap q (= r + i, q = 0..24) the stationary weight
    W_q[e, 7j + r] = conv_ws[j][q - r, e] (zero outside 0 <= q-r < fs_j) and
    the moving operand is emb[:, q :: 7].  This packs all 18 filters x 7 time
    residues = 126 of 128 PE output rows (vs 18/128 for the naive layout).
  * max-over-time: DVE reduce_max over the 290 always-valid columns plus a
    masked (additive -1e30) reduce over the 3 boundary columns, combined into
    a per-core [128, 32] feature tile.
Host (numpy): max over the 7 residues, + conv bias, final 18->2 linear.
"""

import sys

if "/opt/trn_rl_repo" not in sys.path:
    sys.path.insert(0, "/opt/trn_rl_repo")

import numpy as np

# ---- problem constants (hardcoded per harness contract) ----
B, L, E = 256, 2048, 128
VOCAB = 32000
VROWS = VOCAB + 1          # row 32000 is the (zeroed) padding row
FILTER_SIZES = tuple(range(2, 20))   # 18 filters, widths 2..19
NF = len(FILTER_SIZES)
FOLD = 7                   # time positions per output column
NQ = 25                    # taps q = r + i, r<=6, i<=18
NCOL = 293                 # output columns: t = 7n + r covers t <= 2050
NSAFE = 290                # columns 0..289 valid for every (j, r)
NMASK = NCOL - NSAFE       # boundary columns needing per-(j,r) validity mask
NIDX = 2176                # padded tokens per batch row (17 * 128)
IDXC = NIDX // 16          # wrapped index columns
NCORES = 8
BPC = B // NCORES          # batch rows per core


def _pack_weights(conv_ws):
    """[128, NQ*128] fp16 stationary weights; column q*128 + (7*j + r)."""
    wp = np.zeros((E, NQ * 128), dtype=np.float32)
    for j, fs in enumerate(FILTER_SIZES):
        w = np.asarray(conv_ws[j], dtype=np.float32)  # [fs, E]
        for r in range(FOLD):
            m = 7 * j + r
            for i in range(fs):
                q = r + i
                wp[:, q * 128 + m] = w[i]
    return wp.astype(np.float16)


def _make_mask():
    """[128, NMASK] additive f32 mask for columns n = NSAFE..NCOL-1."""
    mask = np.zeros((128, NMASK), dtype=np.float32)
    for j, fs in enumerate(FILTER_SIZES):
        for r in range(FOLD):
            m = 7 * j + r
            for d in range(NMASK):
                t = 7 * (NSAFE + d) + r
                if t > L - fs:
                    mask[m, d] = -1e30
    return mask


def _pack_indices(x_core):
    """x_core [BPC, L] -> [128, BPC*IDXC] int16 wrapped+replicated layout."""
    bpc = x_core.shape[0]
    xp = np.full((bpc, NIDX), VOCAB, dtype=np.int16)
    xp[:, :L] = x_core.astype(np.int16)
    # index i of row b lives at partition i%16, column i//16; replicated x8
    w = xp.reshape(bpc, IDXC, 16).transpose(0, 2, 1)        # [bpc, 16, IDXC]
    w = np.tile(w, (1, 8, 1))                               # [bpc, 128, IDXC]
    return np.ascontiguousarray(
        w.transpose(1, 0, 2).reshape(128, bpc * IDXC))


_NC_CACHE = {}


def _build_nc(bpc=BPC):
    """Build + compile the per-core Bass program (SPMD, same for all cores)."""
    if bpc in _NC_CACHE:
        return _NC_CACHE[bpc]

    import concourse.bacc as bacc
    import concourse.tile as tile
    import concourse.mybir as mybir

    f16, f32, i16 = mybir.dt.float16, mybir.dt.float32, mybir.dt.int16

    nc = bacc.Bacc("TRN2", target_bir_lowering=False, debug=False)
    tab = nc.dram_tensor("tab", [VROWS, E], f16, kind="ExternalInput")
    wpk = nc.dram_tensor("wpk", [E, NQ * 128], f16, kind="ExternalInput")
    msk = nc.dram_tensor("msk", [128, NMASK], f32, kind="ExternalInput")
    idx = nc.dram_tensor("idx", [128, bpc * IDXC], i16, kind="ExternalInput")
    feats = nc.dram_tensor("feats", [128, bpc], f32, kind="ExternalOutput")

    with tile.TileContext(nc) as tc:
        with tc.tile_pool(name="const", bufs=1) as cpool, \
             tc.tile_pool(name="emb", bufs=4) as epool, \
             tc.tile_pool(name="ps", bufs=4, space="PSUM") as pspool, \
             tc.tile_pool(name="red", bufs=4) as rpool:
            wtile = cpool.tile([E, NQ * 128], f16)
            nc.sync.dma_start(wtile[:], wpk[:])
            itile = cpool.tile([128, bpc * IDXC], i16)
            nc.sync.dma_start(itile[:], idx[:])
            mtile = cpool.tile([128, NMASK], f32)
            nc.sync.dma_start(mtile[:], msk[:])
            ftile = cpool.tile([128, bpc], f32)

            for b in range(bpc):
                emb = epool.tile([128, 1, NIDX], f16, tag="emb")
                nc.gpsimd.dma_gather(
                    emb[:], tab[:], itile[:, b * IDXC:(b + 1) * IDXC],
                    NIDX, NIDX, E, transpose=True, single_packet=False)
                embf = emb[:, 0, :]
                ps = pspool.tile([128, NCOL], f32)
                for q in range(NQ):
                    nc.tensor.matmul(
                        ps[:],
                        wtile[:, q * 128:(q + 1) * 128],
                        embf[:, q:q + FOLD * NCOL:FOLD],
                        start=(q == 0), stop=(q == NQ - 1))
                tail = rpool.tile([128, NMASK], f32, tag="tail")
                nc.vector.tensor_add(tail[:], ps[:, NSAFE:], mtile[:])
                m1 = rpool.tile([128, 1], f32, tag="m1")
                nc.vector.reduce_max(m1[:], ps[:, :NSAFE],
                                     axis=mybir.AxisListType.X)
                m2 = rpool.tile([128, 1], f32, tag="m2")
                nc.vector.reduce_max(m2[:], tail[:],
                                     axis=mybir.AxisListType.X)
                nc.vector.tensor_max(ftile[:, b:b + 1], m1[:], m2[:])

            nc.sync.dma_start(feats[:], ftile[:])

    nc.compile()
    _NC_CACHE[bpc] = nc
    return nc


def _host_inputs(x, emb_table, conv_ws):
    """Shared host-side input prep; returns per-core in_maps."""
    x = np.asarray(x)
    tab = np.asarray(emb_table, dtype=np.float32).copy()
    tab[-1] = 0.0                      # nn.Embedding padding_idx row
    tab16 = tab.astype(np.float16)
    wp = _pack_weights(conv_ws)
    mask = _make_mask()
    in_maps = []
    for c in range(NCORES):
        in_maps.append({
            "tab": tab16,
            "wpk": wp,
            "msk": mask,
            "idx": _pack_indices(x[c * BPC:(c + 1) * BPC]),
        })
    return in_maps


def _postprocess(feats_list, conv_bs, lin_w, lin_b):
    """Per-core [128, BPC] feature tiles -> [B, 2] output."""
    fe = np.concatenate(
        [f[:NF * FOLD].reshape(NF, FOLD, -1).max(axis=1) for f in feats_list],
        axis=1)                                   # [18, B]
    fe = fe.T + np.asarray(conv_bs, dtype=np.float32)[None, :]
    out = fe @ np.asarray(lin_w, dtype=np.float32).T \
        + np.asarray(lin_b, dtype=np.float32)[None, :]
    return out.astype(np.float32)


def kernel(x, emb_table, conv_ws, conv_bs, lin_w, lin_b):
    from concourse.bass_utils import run_bass_kernel_spmd

    nc = _build_nc()
    in_maps = _host_inputs(x, emb_table, conv_ws)
    res = run_bass_kernel_spmd(nc, in_maps, core_ids=list(range(NCORES)))
    feats_list = [res.results[c]["feats"] for c in range(NCORES)]
    return _postprocess(feats_list, conv_bs, lin_w, lin_b)


# revision 7
# speedup vs baseline: 1.0114x; 1.0114x over previous
"""Trainium2 Bass kernel for nn_CNNNet (TextCNN: embedding gather -> 18
conv filters (widths 2..19) -> max-over-time -> linear 18->2).

Strategy
--------
Data-parallel over batch: 256 rows -> 32 per NeuronCore x 8 cores.
Per core:
  * fp16 embedding table (padding row 32000 zeroed) stays in HBM; tokens are
    gathered with the SWDGE `dma_gather(transpose=True)` custom instruction,
    which lands each batch-row directly in [128 emb-dims, tokens] SBUF layout
    (the matmul rhs layout) at ~256B/token descriptors across 16 SDMA engines.
  * The 18 convolutions are evaluated as 25 accumulating TensorE matmuls per
    batch row using a time-folded weight packing: output column n covers the
    7 time steps t = 7n + r (r = 0..6); PSUM partition m = 7*j + r holds
    conv_j[7n + r].  For tap q (= r + i, q = 0..24) the stationary weight
    W_q[e, 7j + r] = conv_ws[j][q - r, e] (zero outside 0 <= q-r < fs_j) and
    the moving operand is emb[:, q :: 7].  This packs all 18 filters x 7 time
    residues = 126 of 128 PE output rows (vs 18/128 for the naive layout).
  * max-over-time: DVE reduce_max over the 290 always-valid columns plus a
    masked (additive -1e30) reduce over the 3 boundary columns, combined into
    a per-core [128, 32] feature tile.
Host (numpy): max over the 7 residues, + conv bias, final 18->2 linear.
"""

import sys

if "/opt/trn_rl_repo" not in sys.path:
    sys.path.insert(0, "/opt/trn_rl_repo")

import numpy as np

# ---- problem constants (hardcoded per harness contract) ----
B, L, E = 256, 2048, 128
VOCAB = 32000
VROWS = VOCAB + 1          # row 32000 is the (zeroed) padding row
FILTER_SIZES = tuple(range(2, 20))   # 18 filters, widths 2..19
NF = len(FILTER_SIZES)
FOLD = 7                   # time positions per output column
NQ = 25                    # taps q = r + i, r<=6, i<=18
NCOL = 293                 # output columns: t = 7n + r covers t <= 2050
NSAFE = 290                # columns 0..289 valid for every (j, r)
NMASK = NCOL - NSAFE       # boundary columns needing per-(j,r) validity mask
NIDX = 2176                # padded tokens per batch row (17 * 128)
IDXC = NIDX // 16          # wrapped index columns
NPC = 296                  # tokens per residue class in the SBUF layout
NQUEUES = 1                # SWDGE queues: spread gather desc-gen over Q7 pairs
NCORES = 8
BPC = B // NCORES          # batch rows per core


def _pack_weights(conv_ws):
    """[128, NQ*128] fp16 stationary weights; column q*128 + (7*j + r)."""
    wp = np.zeros((E, NQ * 128), dtype=np.float32)
    for j, fs in enumerate(FILTER_SIZES):
        w = np.asarray(conv_ws[j], dtype=np.float32)  # [fs, E]
        for r in range(FOLD):
            m = 7 * j + r
            for i in range(fs):
                q = r + i
                wp[:, q * 128 + m] = w[i]
    return wp.astype(np.float16)


def _make_mask():
    """[128, NMASK] additive f32 mask for columns n = NSAFE..NCOL-1."""
    mask = np.zeros((128, NMASK), dtype=np.float32)
    for j, fs in enumerate(FILTER_SIZES):
        for r in range(FOLD):
            m = 7 * j + r
            for d in range(NMASK):
                t = 7 * (NSAFE + d) + r
                if t > L - fs:
                    mask[m, d] = -1e30
    return mask


def _pack_indices(x_core):
    """x_core [BPC, L] -> [128, BPC*IDXC] int16 wrapped+replicated layout.

    Tokens are laid out residue-major: gather position p*NPC + n holds token
    7n + p, so the matmul rhs for tap q is the contiguous SBUF slice
    [(q%7)*NPC + q//7, +NCOL).  Positions beyond 7*NPC pad with the zero row.
    """
    bpc = x_core.shape[0]
    xl = np.full((bpc, 7 * NPC), VOCAB, dtype=np.int16)
    t = (7 * np.arange(NPC)[None, :] + np.arange(7)[:, None]).reshape(-1)
    valid = t < L
    xl[:, valid] = x_core.astype(np.int16)[:, t[valid]]
    xp = np.full((bpc, NIDX), VOCAB, dtype=np.int16)
    xp[:, :7 * NPC] = xl
    # index i of row b lives at partition i%16, column i//16; replicated x8
    w = xp.reshape(bpc, IDXC, 16).transpose(0, 2, 1)        # [bpc, 16, IDXC]
    w = np.tile(w, (1, 8, 1))                               # [bpc, 128, IDXC]
    return np.ascontiguousarray(
        w.transpose(1, 0, 2).reshape(128, bpc * IDXC))


_NC_CACHE = {}


def _build_nc(bpc=BPC):
    """Build + compile the per-core Bass program (SPMD, same for all cores)."""
    if bpc in _NC_CACHE:
        return _NC_CACHE[bpc]

    import concourse.bacc as bacc
    import concourse.tile as tile
    import concourse.mybir as mybir

    f16, f32, i16 = mybir.dt.float16, mybir.dt.float32, mybir.dt.int16

    nc = bacc.Bacc("TRN2", target_bir_lowering=False, debug=False,
                   num_swdge_queues=NQUEUES)
    tab = nc.dram_tensor("tab", [VROWS, E], f16, kind="ExternalInput")
    wpk = nc.dram_tensor("wpk", [E, NQ * 128], f16, kind="ExternalInput")
    msk = nc.dram_tensor("msk", [128, NMASK], f32, kind="ExternalInput")
    idx = nc.dram_tensor("idx", [128, bpc * IDXC], i16, kind="ExternalInput")
    feats = nc.dram_tensor("feats", [128, bpc], f32, kind="ExternalOutput")

    with tile.TileContext(nc) as tc:
        with tc.tile_pool(name="const", bufs=1) as cpool, \
             tc.tile_pool(name="emb", bufs=4) as epool, \
             tc.tile_pool(name="ps", bufs=4, space="PSUM") as pspool, \
             tc.tile_pool(name="red", bufs=4) as rpool:
            wtile = cpool.tile([E, NQ * 128], f16)
            nc.sync.dma_start(wtile[:], wpk[:])
            itile = cpool.tile([128, bpc * IDXC], i16)
            nc.sync.dma_start(itile[:], idx[:])
            mtile = cpool.tile([128, NMASK], f32)
            nc.sync.dma_start(mtile[:], msk[:])
            ftile = cpool.tile([128, bpc], f32)

            for b in range(bpc):
                emb = epool.tile([128, 1, NIDX], f16, tag="emb")
                nc.gpsimd.dma_gather(
                    emb[:], tab[:], itile[:, b * IDXC:(b + 1) * IDXC],
                    NIDX, NIDX, E, transpose=True, single_packet=False,
                    queue_num=b % NQUEUES)
                embf = emb[:, 0, :]
                ps = pspool.tile([128, NCOL], f32)
                for q in range(NQ):
                    off = (q % FOLD) * NPC + q // FOLD
                    nc.tensor.matmul(
                        ps[:],
                        wtile[:, q * 128:(q + 1) * 128],
                        embf[:, off:off + NCOL],
                        start=(q == 0), stop=(q == NQ - 1))
                tail = rpool.tile([128, NMASK], f32, tag="tail")
                nc.vector.tensor_add(tail[:], ps[:, NSAFE:], mtile[:])
                m1 = rpool.tile([128, 1], f32, tag="m1")
                nc.vector.reduce_max(m1[:], ps[:, :NSAFE],
                                     axis=mybir.AxisListType.X)
                m2 = rpool.tile([128, 1], f32, tag="m2")
                nc.vector.reduce_max(m2[:], tail[:],
                                     axis=mybir.AxisListType.X)
                nc.vector.tensor_max(ftile[:, b:b + 1], m1[:], m2[:])

            nc.sync.dma_start(feats[:], ftile[:])

    nc.compile()
    _NC_CACHE[bpc] = nc
    return nc


def _host_inputs(x, emb_table, conv_ws):
    """Shared host-side input prep; returns per-core in_maps."""
    x = np.asarray(x)
    tab = np.asarray(emb_table, dtype=np.float32).copy()
    tab[-1] = 0.0                      # nn.Embedding padding_idx row
    tab16 = tab.astype(np.float16)
    wp = _pack_weights(conv_ws)
    mask = _make_mask()
    in_maps = []
    for c in range(NCORES):
        in_maps.append({
            "tab": tab16,
            "wpk": wp,
            "msk": mask,
            "idx": _pack_indices(x[c * BPC:(c + 1) * BPC]),
        })
    return in_maps


def _postprocess(feats_list, conv_bs, lin_w, lin_b):
    """Per-core [128, BPC] feature tiles -> [B, 2] output."""
    fe = np.concatenate(
        [f[:NF * FOLD].reshape(NF, FOLD, -1).max(axis=1) for f in feats_list],
        axis=1)                                   # [18, B]
    fe = fe.T + np.asarray(conv_bs, dtype=np.float32)[None, :]
    out = fe @ np.asarray(lin_w, dtype=np.float32).T \
        + np.asarray(lin_b, dtype=np.float32)[None, :]
    return out.astype(np.float32)


def kernel(x, emb_table, conv_ws, conv_bs, lin_w, lin_b):
    from concourse.bass_utils import run_bass_kernel_spmd

    nc = _build_nc()
    in_maps = _host_inputs(x, emb_table, conv_ws)
    res = run_bass_kernel_spmd(nc, in_maps, core_ids=list(range(NCORES)))
    feats_list = [res.results[c]["feats"] for c in range(NCORES)]
    return _postprocess(feats_list, conv_bs, lin_w, lin_b)
